# revision 22
# baseline (speedup 1.0000x reference)
"""GCNEncoder (3x TransformerConv + GraphNorm + Mish) Trainium2 kernel.

Strategy (8 NeuronCores, SPMD single NEFF):
  - Destination-shard the edges: host sorts edges by dst node; core c owns dst
    nodes [c*6250, (c+1)*6250). Segment softmax + aggregation become PSUM
    accumulation over 128-node destination blocks (no scatter needed).
  - Dense linears replicated: every core computes the full K/V tables (bf16)
    from the full x / h, writes them to its HBM, and gathers per-edge rows with
    indirect DMA. Q/skip linears computed for the own node range only.
  - conv3+conv4 share one fused edge pass (identical structure to conv2's:
    2 "heads" = the two convs).
  - One tiny AllReduce (GraphNorm stats) + one AllGather (post-Mish h) between
    layer 1 and layer 2.

Numerics: bf16 tables/matmuls with fp32 PSUM accumulation; softmax computed
without max-subtraction (alpha in [-7, 7] for this model family; exp is safe in
fp32). Validated ~4e-3 scale-relative absmax error vs the fp32 reference.
"""

import math
import os
import sys
from dataclasses import dataclass, field

import numpy as np

try:
    import ml_dtypes
except ImportError:  # pragma: no cover
    ml_dtypes = None

for _p in ("/opt/trn_rl_repo", "/root/.axon_site/_ro/trn_rl_repo"):
    if os.path.isdir(_p) and _p not in sys.path:
        sys.path.append(_p)

BF16 = ml_dtypes.bfloat16


@dataclass
class Cfg:
    N: int = 50000          # nodes
    E: int = 800000         # edges
    IN: int = 256           # input feature dim
    C: int = 64             # out channels per head
    ncores: int = 8
    BLK: int = 128          # dst nodes per PSUM block

    @property
    def NPC(self):          # nodes per core
        return self.N // self.ncores

    @property
    def NB(self):           # dst blocks per core
        return (self.NPC + self.BLK - 1) // self.BLK

    @property
    def NPAD(self):         # padded nodes per core
        return self.NB * self.BLK

    @property
    def NALL(self):         # padded nodes total (blocked node space)
        return self.ncores * self.NPAD


# ---------------------------------------------------------------------------
# Host-side prep: edge sorting/sharding + input marshalling (layout only).
# ---------------------------------------------------------------------------

def _np(a):
    return np.asarray(a)


def host_prep(cfg: Cfg, x, edge_index, params):
    N, E, IN, C = cfg.N, cfg.E, cfg.IN, cfg.C
    NPC, BLK, NB, NPAD = cfg.NPC, cfg.BLK, cfg.NB, cfg.NPAD

    x = _np(x).astype(np.float32)
    ei = _np(edge_index)
    src = ei[0].astype(np.int64)
    dst = ei[1].astype(np.int64)

    order = np.argsort(dst, kind="stable")
    ss, ds = src[order], dst[order]
    core_arr = ds // NPC
    loc = ds - core_arr * NPC
    gblk = core_arr * NB + loc // BLK   # non-decreasing
    blk_lo = np.searchsorted(gblk, np.arange(cfg.ncores * NB), side="left")
    blk_hi = np.searchsorted(gblk, np.arange(cfg.ncores * NB), side="right")

    SPLIT = min(32768, cfg.NALL)  # int16 gather-index limit

    def blocked_id(n):
        c = n // NPC
        return c * NPAD + (n - c * NPC)

    # per (core, block): edge lists split into A (blocked src < SPLIT) and B
    eA, eB = {}, {}
    nA = np.zeros((cfg.ncores, NB), np.int64)
    nB = np.zeros((cfg.ncores, NB), np.int64)
    for c in range(cfg.ncores):
        for b in range(NB):
            lo, hi = blk_lo[c * NB + b], blk_hi[c * NB + b]
            sb = blocked_id(ss[lo:hi])
            dd = ds[lo:hi]
            mA = sb < SPLIT
            eA[c, b] = (sb[mA], dd[mA])
            eB[c, b] = (sb[~mA] - SPLIT, dd[~mA])
            nA[c, b] = mA.sum()
            nB[c, b] = (~mA).sum()

    MAs = [int(math.ceil(nA[:, b].max() / BLK)) for b in range(NB)]
    MBs = [int(math.ceil(nB[:, b].max() / BLK)) for b in range(NB)]
    Ms = [max(1, MAs[b] + MBs[b]) for b in range(NB)]
    S = int(sum(Ms))
    moff = np.concatenate([[0], np.cumsum(Ms)[:-1]]).astype(np.int64)
    W16 = int(sum((MAs[b] + MBs[b] + Ms[b]) * 8 for b in range(NB)))

    def wrap16(vals, n_slots):
        """index list -> [128, n_slots/16] int16 (wrapped, group-replicated)"""
        L = np.zeros(n_slots, np.int64)
        L[:len(vals)] = vals
        w = L.reshape(n_slots // 16, 16).T.astype(np.int16)   # [16, n/16]
        return np.tile(w, (8, 1))

    per_core = []
    for c in range(cfg.ncores):
        idx16 = np.zeros((BLK, W16), np.int16)
        rel = np.full((BLK, S), float(BLK), np.float32)
        qloc = np.zeros((BLK, S), np.float64)
        off = 0
        for b in range(NB):
            MA, MB, M = MAs[b], MBs[b], Ms[b]
            sA, dA = eA[c, b]
            sB, dB = eB[c, b]
            if MA:
                idx16[:, off:off + MA * 8] = wrap16(sA, MA * BLK)
                off += MA * 8
            if MB:
                idx16[:, off:off + MB * 8] = wrap16(sB, MB * BLK)
                off += MB * 8
            # q indices + dst_rel follow the same slot order: A then B
            dq = np.concatenate([dA, dB])
            qv = np.zeros(M * BLK, np.int64)
            rv = np.full(M * BLK, float(BLK), np.float32)
            # section A at chunks [0, MA), B at [MA, MA+MB)
            qv[:len(dA)] = dA - c * NPC
            rv[:len(dA)] = dA - (c * NPC + b * BLK)
            if MB:
                qv[MA * BLK:MA * BLK + len(dB)] = dB - c * NPC
                rv[MA * BLK:MA * BLK + len(dB)] = dB - (c * NPC + b * BLK)
            idx16[:, off:off + M * 8] = wrap16(qv, M * BLK)
            off += M * 8
            sl = slice(moff[b], moff[b] + M)
            rel[:, sl] = rv.reshape(M, BLK).T
        assert off == W16
        per_core.append({
            "eidx16": idx16,                  # [128, W16] i16
            "erel": rel.astype(BF16),         # [128, S]
        })

    # x transposed into blocked node space (layout marshalling)
    xT_blk = np.zeros((IN, cfg.NALL), BF16)
    xb = x.astype(BF16)
    for c in range(cfg.ncores):
        xT_blk[:, c * NPAD:c * NPAD + NPC] = xb[c * NPC:(c + 1) * NPC].T
    for c in range(cfg.ncores):
        per_core[c]["xT_own"] = np.ascontiguousarray(
            xT_blk[:, c * NPAD:(c + 1) * NPAD])

    p2, p3, p4 = params["conv2"], params["conv3"], params["conv4"]
    g = params["gn"]
    f32 = np.float32

    def cc(*arrs):
        return np.concatenate([_np(a).astype(f32) for a in arrs], axis=-1)

    shared = {
        "xT_blk": xT_blk,
        "Wx_kv": cc(p2["Wk"], p2["Wv"], p4["Wk"], p4["Wv"]).astype(BF16),
        "bx_kv": cc(p2["bk"], p2["bv"], p4["bk"], p4["bv"])[None].astype(BF16),
        "Wh_kv": cc(p3["Wk"], p3["Wv"]).astype(BF16),
        "bh_kv": cc(p3["bk"], p3["bv"])[None].astype(BF16),
        "Wx_own": cc(p2["Wq"], p2["Ws"], p4["Wq"], p4["Ws"]).astype(BF16),
        "bx_own": cc(p2["bq"], p2["bs"], p4["bq"], p4["bs"])[None].astype(BF16),
        "Wh_own": cc(p3["Wq"], p3["Ws"]).astype(BF16),
        "bh_own": cc(p3["bq"], p3["bs"])[None].astype(BF16),
        "gn_row": cc(g["weight"], g["bias"], g["mean_scale"])[None].astype(f32),
        "iota_row": np.tile(np.arange(BLK, dtype=f32), (BLK, 1)).astype(BF16),
        "vmask_in": (np.arange(BLK) < (NPC - (NB - 1) * BLK)
                     ).astype(f32)[:, None],
    }
    meta = {"Ms": Ms, "MAs": MAs, "MBs": MBs, "S": S, "W16": W16,
            "SPLIT": SPLIT}
    return shared, per_core, meta


# ---------------------------------------------------------------------------
# Kernel builder
# ---------------------------------------------------------------------------

def build(cfg: Cfg, meta, debug=False):
    import concourse.bacc as bacc
    import concourse.bass as bass
    import concourse.tile as tile
    import concourse.mybir as mybir
    from concourse.bass import IndirectOffsetOnAxis

    dt = mybir.dt
    Alu = mybir.AluOpType
    Act = mybir.ActivationFunctionType

    N, IN, C = cfg.N, cfg.IN, cfg.C
    BLK, NB, NPAD, NALL = cfg.BLK, cfg.NB, cfg.NPAD, cfg.NALL
    H2 = 2 * C      # 128: conv2 per-node q/k/v width (2 heads)
    KV = 2 * H2     # 256: kv table row width
    NCOL = H2 + 2   # 130: [wv(128) | p(2)] matmul rhs width
    Ms, MAs, MBs = meta["Ms"], meta["MAs"], meta["MBs"]
    S, W16, SPLIT = meta["S"], meta["W16"], meta["SPLIT"]
    moff = np.concatenate([[0], np.cumsum(Ms)[:-1]]).astype(np.int64)
    # per-block offsets into the wrapped int16 index tensor
    i16off = []
    off = 0
    for b in range(NB):
        i16off.append(off)
        off += (MAs[b] + MBs[b] + Ms[b]) * 8
    assert off == W16

    nc = bacc.Bacc("TRN2", target_bir_lowering=False, debug=False,
                   enable_asserts=False, num_devices=cfg.ncores)

    # ---- I/O ----
    f32, bf16, i32 = dt.float32, dt.bfloat16, dt.int32
    xT_blk = nc.dram_tensor("xT_blk", [IN, NALL], bf16, kind="ExternalInput")
    xT_own = nc.dram_tensor("xT_own", [IN, NPAD], bf16, kind="ExternalInput")
    eidx16 = nc.dram_tensor("eidx16", [BLK, W16], dt.int16, kind="ExternalInput")
    erel = nc.dram_tensor("erel", [BLK, S], bf16, kind="ExternalInput")
    Wx_kv = nc.dram_tensor("Wx_kv", [IN, 384], bf16, kind="ExternalInput")
    bx_kv = nc.dram_tensor("bx_kv", [1, 384], bf16, kind="ExternalInput")
    Wh_kv = nc.dram_tensor("Wh_kv", [H2, H2], bf16, kind="ExternalInput")
    bh_kv = nc.dram_tensor("bh_kv", [1, H2], bf16, kind="ExternalInput")
    Wx_own = nc.dram_tensor("Wx_own", [IN, 384], bf16, kind="ExternalInput")
    bx_own = nc.dram_tensor("bx_own", [1, 384], bf16, kind="ExternalInput")
    Wh_own = nc.dram_tensor("Wh_own", [H2, H2], bf16, kind="ExternalInput")
    bh_own = nc.dram_tensor("bh_own", [1, H2], bf16, kind="ExternalInput")
    gn_row = nc.dram_tensor("gn_row", [1, 3 * H2], f32, kind="ExternalInput")
    iota_in = nc.dram_tensor("iota_row", [BLK, BLK], bf16, kind="ExternalInput")
    vmask_in = nc.dram_tensor("vmask_in", [BLK, 1], f32, kind="ExternalInput")
    out_t = nc.dram_tensor("out", [NPAD, C], f32, kind="ExternalOutput")
    if debug:
        dbg_kv2 = nc.dram_tensor("dbg_kv2", [BLK, KV], bf16, kind="ExternalOutput")
        dbg_q2 = nc.dram_tensor("dbg_q2", [BLK, H2], bf16, kind="ExternalOutput")
        dbg_h1 = nc.dram_tensor("dbg_h1", [BLK, NB * H2], f32, kind="ExternalOutput")
        dbg_st = nc.dram_tensor("dbg_st", [1, KV], f32, kind="ExternalOutput")
        dbg_hf = nc.dram_tensor("dbg_hf", [2 * BLK, H2], bf16, kind="ExternalOutput")
        dbg_kv34 = nc.dram_tensor("dbg_kv34", [BLK, KV], bf16, kind="ExternalOutput")
        dbg_q34 = nc.dram_tensor("dbg_q34", [BLK, H2], bf16, kind="ExternalOutput")

    # ---- internal DRAM ----
    kv2_t = nc.dram_tensor("kv2_tbl", [NALL, KV], bf16, kind="Internal")
    kv34_t = nc.dram_tensor("kv34_tbl", [NALL, KV], bf16, kind="Internal")
    q2o_t = nc.dram_tensor("q2_own", [NPAD, H2], bf16, kind="Internal")
    q34o_t = nc.dram_tensor("q34_own", [NPAD, H2], bf16, kind="Internal")
    h_own_t = nc.dram_tensor("h_own", [NPAD, H2], bf16, kind="Internal")
    shared_sp = "Shared" if cfg.ncores > 4 else "Local"
    h_full_t = nc.dram_tensor("h_full", [NALL, H2], bf16, kind="Internal",
                              addr_space=shared_sp)
    st_in_t = nc.dram_tensor("stats_in", [1, KV], f32, kind="Internal")
    st_out_t = nc.dram_tensor("stats_out", [1, KV], f32, kind="Internal",
                              addr_space=shared_sp)

    with tile.TileContext(nc) as tc:
        with (
            tc.tile_pool(name="persist", bufs=1) as pp,
            tc.tile_pool(name="dense", bufs=3) as dp,
            tc.tile_pool(name="edge", bufs=2) as ep,
            tc.tile_pool(name="small", bufs=2) as sp,
            tc.tile_pool(name="once", bufs=1) as op_,
            tc.tile_pool(name="dpsum", bufs=3, space="PSUM") as dps,
            tc.tile_pool(name="epsum", bufs=2, space="PSUM") as eps,
            tc.tile_pool(name="spsum", bufs=1, space="PSUM") as sps,
        ):
            # ---- persistent SBUF ----
            s2_own = pp.tile([BLK, NB * H2], f32)      # conv2 skip (own)
            s34_own = pp.tile([BLK, NB * C], f32)      # conv3+conv4 skip (own)
            h1_own = pp.tile([BLK, NB * H2], f32)      # conv2 output (own)
            hmish = pp.tile([BLK, NB * H2], bf16)      # post-norm/mish (own)
            out_sb = pp.tile([BLK, NB * C], f32)       # final output (own)
            eidx_sb = pp.tile([BLK, W16], dt.int16)
            erel_sb = pp.tile([BLK, S], bf16)
            wxkv_sb = pp.tile([BLK, 2 * 384], bf16)    # 2 K-halves side by side
            wxown_sb = pp.tile([BLK, 2 * 384], bf16)
            whkv_sb = pp.tile([BLK, H2], bf16)
            whown_sb = pp.tile([BLK, H2], bf16)
            bias_sb = pp.tile([1, 384 * 2 + H2 * 2], bf16)
            gn_sb = pp.tile([1, 3 * H2], f32)
            iota_sb = pp.tile([BLK, BLK], bf16)
            ones_row = pp.tile([1, BLK], bf16)         # K=1 bias matmul lhsT
            ones_col = pp.tile([BLK, 1], f32)          # stats matmul lhsT
            vmask = pp.tile([BLK, 1], f32)             # last-block valid rows
            affine = pp.tile([1, 2 * H2], f32)         # graphnorm a | b row
            affine_full = pp.tile([BLK, 2 * H2], f32)  # row replicated

            # ---- constants / input staging ----
            nc.sync.dma_start(eidx_sb[:], eidx16[:])
            nc.sync.dma_start(erel_sb[:], erel[:])
            nc.sync.dma_start(wxkv_sb[:, 0:384], Wx_kv[0:BLK, :])
            nc.sync.dma_start(wxkv_sb[:, 384:768], Wx_kv[BLK:IN, :])
            nc.sync.dma_start(wxown_sb[:, 0:384], Wx_own[0:BLK, :])
            nc.sync.dma_start(wxown_sb[:, 384:768], Wx_own[BLK:IN, :])
            nc.sync.dma_start(whkv_sb[:], Wh_kv[:])
            nc.sync.dma_start(whown_sb[:], Wh_own[:])
            nc.sync.dma_start(bias_sb[:, 0:384], bx_kv[:])
            nc.sync.dma_start(bias_sb[:, 384:768], bx_own[:])
            nc.sync.dma_start(bias_sb[:, 768:768 + H2], bh_kv[:])
            nc.sync.dma_start(bias_sb[:, 768 + H2:768 + 2 * H2], bh_own[:])
            nc.sync.dma_start(gn_sb[:], gn_row[:])
            nc.sync.dma_start(iota_sb[:], iota_in[:])
            nc.vector.memset(ones_row[:], 1.0)
            nc.vector.memset(ones_col[:], 1.0)
            nc.sync.dma_start(vmask[:], vmask_in[:])

            b_xkv = bias_sb[:, 0:384]
            b_xown = bias_sb[:, 384:768]
            b_hkv = bias_sb[:, 768:768 + H2]
            b_hown = bias_sb[:, 768 + H2:768 + 2 * H2]

            # ---------------------------------------------------------------
            # Dense pass over x (all nodes): kv2 rows + kv34[k4|v4] half.
            # lhsT = xT tile [128 feat, 128 nodes]; rhs = W [128 feat, 384].
            # ---------------------------------------------------------------
            nblk_all = NALL // BLK
            SUP = 4  # node-tiles per DMA load

            def x_dense(xT_src, n_tiles, W_sb, b_row, sink):
                for st in range(0, n_tiles, SUP):
                    nsub = min(SUP, n_tiles - st)
                    w = nsub * BLK
                    xt = dp.tile([BLK, 2 * SUP * BLK], bf16, tag="xt")
                    nc.sync.dma_start(xt[:, 0:w],
                                      xT_src[0:BLK, st * BLK:st * BLK + w])
                    nc.sync.dma_start(xt[:, SUP * BLK:SUP * BLK + w],
                                      xT_src[BLK:IN, st * BLK:st * BLK + w])
                    for s in range(nsub):
                        t = st + s
                        ps = dps.tile([BLK, 384], f32, space="PSUM", tag="dps")
                        nc.tensor.matmul(ps[:], lhsT=xt[:, s * BLK:(s + 1) * BLK],
                                         rhs=W_sb[:, 0:384],
                                         start=True, stop=False)
                        nc.tensor.matmul(
                            ps[:],
                            lhsT=xt[:, SUP * BLK + s * BLK:SUP * BLK + (s + 1) * BLK],
                            rhs=W_sb[:, 384:768], start=False, stop=False)
                        nc.tensor.matmul(ps[:], lhsT=ones_row[:1, 0:BLK],
                                         rhs=b_row, start=False, stop=True)
                        sink(t, ps)

            def kv_x_sink(t, ps):
                ot = dp.tile([BLK, 384], bf16, tag="dout")
                nc.vector.tensor_copy(ot[:], ps[:])
                nc.sync.dma_start(kv2_t[t * BLK:(t + 1) * BLK, :], ot[:, 0:KV])
                nc.sync.dma_start(kv34_t[t * BLK:(t + 1) * BLK, H2:KV],
                                  ot[:, KV:384])

            x_dense(xT_blk, nblk_all, wxkv_sb, b_xkv[:1, :], kv_x_sink)

            # ---- own dense pass over x: q2 | s2 | q4 | s4 ----
            def own_x_sink(b, ps):
                qb = dp.tile([BLK, H2 + C], bf16, tag="qbf")
                nc.vector.tensor_copy(qb[:, 0:H2], ps[:, 0:H2])
                nc.vector.tensor_copy(qb[:, H2:H2 + C], ps[:, KV:KV + C])
                nc.sync.dma_start(q2o_t[b * BLK:(b + 1) * BLK, :], qb[:, 0:H2])
                nc.sync.dma_start(q34o_t[b * BLK:(b + 1) * BLK, C:H2],
                                  qb[:, H2:H2 + C])
                nc.vector.tensor_copy(s2_own[:, b * H2:(b + 1) * H2],
                                      ps[:, H2:KV])
                nc.vector.tensor_copy(s34_own[:, b * C:(b + 1) * C],
                                      ps[:, KV + C:384])

            x_dense(xT_own, NB, wxown_sb, b_xown[:1, :], own_x_sink)

            # ---------------------------------------------------------------
            # Edge pass (shared for conv2 and conv34)
            # ---------------------------------------------------------------
            def edge_pass(kv_tbl, q_tbl, conv2_layout, epilogue):
                for b in range(NB):
                    M, MA, MB = int(Ms[b]), int(MAs[b]), int(MBs[b])
                    mo = int(moff[b])
                    io = i16off[b]
                    kv_t_sb = ep.tile([BLK, M * KV], bf16, tag="kvg")
                    qg = ep.tile([BLK, M * H2], bf16, tag="qg")
                    if MA:
                        nc.gpsimd.dma_gather(
                            kv_t_sb[:, 0:MA * KV].rearrange(
                                "p (m w) -> p m w", w=KV),
                            kv_tbl[0:SPLIT, :],
                            eidx_sb[:, io:io + MA * 8],
                            MA * BLK, MA * BLK, KV, single_packet=False)
                        io += MA * 8
                    if MB:
                        nc.gpsimd.dma_gather(
                            kv_t_sb[:, MA * KV:M * KV].rearrange(
                                "p (m w) -> p m w", w=KV),
                            kv_tbl[SPLIT:NALL, :],
                            eidx_sb[:, io:io + MB * 8],
                            MB * BLK, MB * BLK, KV, single_packet=False)
                        io += MB * 8
                    nc.gpsimd.dma_gather(
                        qg[:].rearrange("p (m w) -> p m w", w=H2),
                        q_tbl[:],
                        eidx_sb[:, io:io + M * 8],
                        M * BLK, M * BLK, H2, single_packet=False)
                    kv4 = kv_t_sb[:].rearrange("p (m w) -> p m w", w=KV)
                    if conv2_layout:
                        # kv row = [k(2x64) | v(2x64)] (head stride C)
                        k_ap = kv4[:, :, 0:H2].rearrange(
                            "p m (h c) -> p m h c", c=C)
                        v_ap = kv4[:, :, H2:KV].rearrange(
                            "p m (h c) -> p m h c", c=C)
                    else:
                        # kv row = [k3|v3|k4|v4] (head stride H2)
                        kvh = kv4.rearrange("p m (h g) -> p m h g", h=2)
                        k_ap = kvh[:, :, :, 0:C]
                        v_ap = kvh[:, :, :, C:H2]
                    # prod = q * k  -> [128, M, 2, C]
                    prod = ep.tile([BLK, M * H2], bf16, tag="prod")
                    nc.vector.tensor_mul(
                        prod[:].rearrange("p (m h c) -> p m h c", h=2, c=C),
                        qg[:].rearrange("p (m h c) -> p m h c", h=2, c=C),
                        k_ap)
                    # alpha = rowsum over C -> [128, 2M] f32
                    alpha = ep.tile([BLK, M * 2], f32, tag="alpha")
                    nc.vector.reduce_sum(
                        alpha[:],
                        prod[:].rearrange("p (mh c) -> p mh c", c=C),
                        axis=mybir.AxisListType.X)
                    # p = exp(alpha/8) into wvp[:, :, 128:130]
                    wvp = ep.tile([BLK, M * NCOL], bf16, tag="wvp")
                    wvp3 = wvp[:].rearrange("p (m w) -> p m w", w=NCOL)
                    nc.scalar.activation(wvp3[:, :, H2:NCOL], alpha[:].rearrange(
                        "p (m h) -> p m h", h=2), Act.Exp,
                        scale=float(1.0 / math.sqrt(C)))
                    # wv = v * p
                    p_ap = wvp3[:, :, H2:NCOL].unsqueeze(3).broadcast_to(
                        [BLK, M, 2, C])
                    nc.vector.tensor_mul(
                        wvp3[:, :, 0:H2].rearrange("p m (h c) -> p m h c", c=C),
                        v_ap, p_ap)
                    # onehot[e, n] = (dst_rel[e] == n)
                    oh = ep.tile([BLK, M * BLK], bf16, tag="oh")
                    rel_ap = erel_sb[:, mo:mo + M].unsqueeze(2).broadcast_to(
                        [BLK, M, BLK])
                    io_ap = iota_sb[:].unsqueeze(1).broadcast_to(
                        [BLK, M, BLK])
                    nc.vector.tensor_tensor(
                        oh[:].rearrange("p (m n) -> p m n", n=BLK),
                        rel_ap, io_ap, op=Alu.is_equal)
                    # segment accumulate: psum[n, :] += onehot^T @ [wv | p]
                    ps = eps.tile([BLK, NCOL], f32, space="PSUM", tag="eps")
                    for j in range(M):
                        nc.tensor.matmul(
                            ps[:], lhsT=oh[:, j * BLK:(j + 1) * BLK],
                            rhs=wvp[:, j * NCOL:(j + 1) * NCOL],
                            start=(j == 0), stop=(j == M - 1))
                    epilogue(b, ps)

            # ---- conv2 edge pass ----
            st_ps = sps.tile([1, H2], f32, space="PSUM", tag="sth")
            st_ps2 = sps.tile([1, H2], f32, space="PSUM", tag="stsq")

            def epi_conv2(b, ps):
                den = sp.tile([BLK, 2], f32, tag="den")
                nc.vector.tensor_scalar_add(den[:], ps[:, H2:NCOL], 1e-16)
                rden = sp.tile([BLK, 2], f32, tag="rden")
                nc.vector.reciprocal(rden[:], den[:])
                h1b = h1_own[:, b * H2:(b + 1) * H2]
                rd_ap = rden[:].unsqueeze(2).broadcast_to([BLK, 2, C])
                nc.vector.tensor_mul(
                    h1b.rearrange("p (h c) -> p h c", c=C),
                    ps[:, 0:H2].rearrange("p (h c) -> p h c", c=C), rd_ap)
                nc.vector.tensor_add(h1b, h1b, s2_own[:, b * H2:(b + 1) * H2])
                if b == NB - 1:
                    nc.vector.tensor_scalar_mul(h1b, h1b, vmask[:, 0:1])
                sq = sp.tile([BLK, H2], f32, tag="sq")
                nc.scalar.square(sq[:], h1b)
                nc.tensor.matmul(st_ps[0:1, :], lhsT=ones_col[:, 0:1],
                                 rhs=h1b, start=(b == 0), stop=(b == NB - 1))
                nc.tensor.matmul(st_ps2[0:1, :], lhsT=ones_col[:, 0:1],
                                 rhs=sq[:], start=(b == 0), stop=(b == NB - 1))

            edge_pass(kv2_t, q2o_t, conv2_layout=True, epilogue=epi_conv2)

            # ---- GraphNorm stats allreduce + affine ----
            st_sb = op_.tile([1, KV], f32, tag="st")
            nc.vector.tensor_copy(st_sb[:1, 0:H2], st_ps[:])
            nc.vector.tensor_copy(st_sb[:1, H2:KV], st_ps2[:])
            nc.sync.dma_start(st_in_t[:], st_sb[:])
            nc.gpsimd.collective_compute(
                "AllReduce", Alu.add,
                replica_groups=[list(range(cfg.ncores))],
                ins=[st_in_t[:].opt()], outs=[st_out_t[:].opt()])
            stg = op_.tile([1, KV], f32, tag="stg")
            nc.sync.dma_start(stg[:], st_out_t[:])
            mean = op_.tile([1, H2], f32, tag="mean")
            nc.vector.tensor_scalar_mul(mean[:], stg[:1, 0:H2], 1.0 / N)
            e2 = op_.tile([1, H2], f32, tag="e2")
            nc.vector.tensor_scalar_mul(e2[:], stg[:1, H2:KV], 1.0 / N)
            m2 = op_.tile([1, H2], f32, tag="m2")
            nc.vector.tensor_mul(m2[:], mean[:], gn_sb[:1, 2 * H2:3 * H2])
            # var = e2 - 2*mean*m2 + m2^2
            var = op_.tile([1, H2], f32, tag="var")
            t1 = op_.tile([1, H2], f32, tag="t1")
            nc.vector.tensor_mul(t1[:], mean[:], m2[:])
            nc.vector.scalar_tensor_tensor(var[:], t1[:], -2.0, e2[:],
                                           op0=Alu.mult, op1=Alu.add)
            nc.vector.tensor_mul(t1[:], m2[:], m2[:])
            nc.vector.tensor_add(var[:], var[:], t1[:])
            nc.vector.tensor_scalar_add(var[:], var[:], 1e-5)
            sd = op_.tile([1, H2], f32, tag="sd")
            nc.scalar.activation(sd[:], var[:], Act.Sqrt)
            rsd = op_.tile([1, H2], f32, tag="rsd")
            nc.vector.reciprocal(rsd[:], sd[:])
            nc.vector.tensor_mul(affine[:1, 0:H2], rsd[:],
                                 gn_sb[:1, 0:H2])             # a
            nc.vector.tensor_mul(t1[:], affine[:1, 0:H2], m2[:])
            nc.vector.tensor_sub(affine[:1, H2:KV],
                                 gn_sb[:1, H2:KV], t1[:])     # b

            # ---- normalize + mish over own nodes ----
            ones_f32 = op_.tile([1, BLK], f32, tag="of")
            nc.vector.memset(ones_f32[:], 1.0)
            ps_aff = dps.tile([BLK, 384], f32, space="PSUM", tag="dps")
            nc.tensor.matmul(ps_aff[:, 0:KV], lhsT=ones_f32[:1, 0:BLK],
                             rhs=affine[:1, :], start=True, stop=True)
            nc.vector.tensor_copy(affine_full[:], ps_aff[:, 0:KV])
            a_ap = affine_full[:, 0:H2].unsqueeze(1).broadcast_to(
                [BLK, NB, H2])
            b_ap = affine_full[:, H2:KV].unsqueeze(1).broadcast_to(
                [BLK, NB, H2])
            h3 = h1_own[:].rearrange("p (b w) -> p b w", w=H2)
            nc.vector.tensor_mul(h3, h3, a_ap)
            nc.vector.tensor_add(h3, h3, b_ap)
            # mish(x) = x * tanh(ln(1 + exp(x)))  (no Mish/Softplus LUT on HW)
            nc.scalar.activation(hmish[:], h1_own[:], Act.Exp)
            nc.scalar.activation(hmish[:], hmish[:], Act.Ln, bias=1.0)
            nc.scalar.activation(hmish[:], hmish[:], Act.Tanh)
            nc.vector.tensor_tensor(hmish[:], h1_own[:], hmish[:], op=Alu.mult)
            nc.vector.tensor_scalar_mul(
                hmish[:, (NB - 1) * H2:NB * H2],
                hmish[:, (NB - 1) * H2:NB * H2], vmask[:, 0:1])
            # write h_own (blocked node-major) + allgather
            nc.sync.dma_start(
                h_own_t[:].rearrange("(b p) w -> p b w", p=BLK),
                hmish[:].rearrange("p (b w) -> p b w", w=H2))
            nc.gpsimd.collective_compute(
                "AllGather", Alu.bypass,
                replica_groups=[list(range(cfg.ncores))],
                ins=[h_own_t[:].opt()], outs=[h_full_t[:].opt()])

            # ---- own dense pass over h: q3 | s3 ----
            for b in range(NB):
                ht = dp.tile([BLK, BLK], bf16, tag="ht")
                nc.sync.dma_start_transpose(
                    ht[:], h_own_t[b * BLK:(b + 1) * BLK, :])
                ps = dps.tile([BLK, 384], f32, space="PSUM", tag="dps")
                nc.tensor.matmul(ps[:, 0:H2], lhsT=ht[:], rhs=whown_sb[:],
                                 start=True, stop=False)
                nc.tensor.matmul(ps[:, 0:H2], lhsT=ones_row[:1, 0:BLK],
                                 rhs=b_hown[:1, :], start=False, stop=True)
                qb = dp.tile([BLK, H2 + C], bf16, tag="qbf")
                nc.vector.tensor_copy(qb[:, 0:C], ps[:, 0:C])
                nc.sync.dma_start(q34o_t[b * BLK:(b + 1) * BLK, 0:C],
                                  qb[:, 0:C])
                nc.vector.tensor_add(s34_own[:, b * C:(b + 1) * C],
                                     s34_own[:, b * C:(b + 1) * C],
                                     ps[:, C:H2])

            # ---- dense pass over h (all nodes): kv34 [k3|v3] half ----
            for t in range(nblk_all):
                ht = dp.tile([BLK, BLK], bf16, tag="ht")
                nc.sync.dma_start_transpose(
                    ht[:], h_full_t[t * BLK:(t + 1) * BLK, :])
                ps = dps.tile([BLK, 384], f32, space="PSUM", tag="dps")
                nc.tensor.matmul(ps[:, 0:H2], lhsT=ht[:], rhs=whkv_sb[:],
                                 start=True, stop=False)
                nc.tensor.matmul(ps[:, 0:H2], lhsT=ones_row[:1, 0:BLK],
                                 rhs=b_hkv[:1, :], start=False, stop=True)
                ot = dp.tile([BLK, 384], bf16, tag="dout")
                nc.vector.tensor_copy(ot[:, 0:H2], ps[:, 0:H2])
                nc.sync.dma_start(kv34_t[t * BLK:(t + 1) * BLK, 0:H2],
                                  ot[:, 0:H2])

            # ---- conv3+conv4 fused edge pass ----
            def epi_conv34(b, ps):
                den = sp.tile([BLK, 2], f32, tag="den")
                nc.vector.tensor_scalar_add(den[:], ps[:, H2:NCOL], 1e-16)
                rden = sp.tile([BLK, 2], f32, tag="rden")
                nc.vector.reciprocal(rden[:], den[:])
                tt = sp.tile([BLK, H2], f32, tag="tt")
                rd_ap = rden[:].unsqueeze(2).broadcast_to([BLK, 2, C])
                nc.vector.tensor_mul(
                    tt[:].rearrange("p (h c) -> p h c", c=C),
                    ps[:, 0:H2].rearrange("p (h c) -> p h c", c=C), rd_ap)
                ob = out_sb[:, b * C:(b + 1) * C]
                nc.vector.tensor_add(ob, tt[:, 0:C], tt[:, C:H2])
                nc.vector.tensor_add(ob, ob, s34_own[:, b * C:(b + 1) * C])

            edge_pass(kv34_t, q34o_t, conv2_layout=False, epilogue=epi_conv34)

            if debug:
                nc.sync.dma_start(dbg_kv2[:], kv2_t[0:BLK, :])
                nc.sync.dma_start(dbg_q2[:], q2o_t[0:BLK, :])
                nc.sync.dma_start(dbg_h1[:], h1_own[:])
                nc.sync.dma_start(dbg_st[:], stg[:])
                nc.sync.dma_start(dbg_hf[:], h_full_t[0:2 * BLK, :])
                nc.sync.dma_start(dbg_kv34[:], kv34_t[0:BLK, :])
                nc.sync.dma_start(dbg_q34[:], q34o_t[0:BLK, :])
            # ---- final output ----
            nc.sync.dma_start(
                out_t[:].rearrange("(b p) w -> p b w", p=BLK),
                out_sb[:].rearrange("p (b w) -> p b w", w=C))

    nc.compile()
    return nc


# ---------------------------------------------------------------------------
# Entry point
# ---------------------------------------------------------------------------

def kernel(x=None, edge_index=None, params=None, _bench=False, **kw):
    if x is None:
        x = kw["x"]
    if edge_index is None:
        edge_index = kw["edge_index"]
    if params is None:
        params = kw["params"]
    cfg = Cfg()
    params = {k: {kk: _np(vv) for kk, vv in v.items()}
              for k, v in params.items()}
    shared, per_core, meta = host_prep(cfg, x, edge_index, params)
    nc = build(cfg, meta)

    from concourse import bass_utils
    in_maps = []
    for c in range(cfg.ncores):
        m = dict(shared)
        m.update(per_core[c])
        in_maps.append(m)
    res = bass_utils.run_bass_kernel_spmd(
        nc, in_maps, core_ids=list(range(cfg.ncores)))
    outs = [res.results[c]["out"][:cfg.NPC] for c in range(cfg.ncores)]
    full = np.concatenate(outs, axis=0).astype(np.float32)
    if _bench:
        return full, res
    return full


if __name__ == "__main__":
    import reference
    inputs = reference.setup_inputs()
    out = kernel(**{k: v for k, v in inputs.items()})
    print("out", out.shape, out.dtype)


# revision 26
# speedup vs baseline: 1.0302x; 1.0302x over previous
"""GCNEncoder (3x TransformerConv + GraphNorm + Mish) Trainium2 kernel.

Strategy (8 NeuronCores, SPMD single NEFF):
  - Destination-shard the edges: host sorts edges by dst node; core c owns dst
    nodes [c*6250, (c+1)*6250). Segment softmax + aggregation become PSUM
    accumulation over 128-node destination blocks (no scatter needed).
  - Dense linears replicated: every core computes the full K/V tables (bf16)
    from the full x / h, writes them to its HBM, and gathers per-edge rows with
    indirect DMA. Q/skip linears computed for the own node range only.
  - conv3+conv4 share one fused edge pass (identical structure to conv2's:
    2 "heads" = the two convs).
  - One tiny AllReduce (GraphNorm stats) + one AllGather (post-Mish h) between
    layer 1 and layer 2.

Numerics: bf16 tables/matmuls with fp32 PSUM accumulation; softmax computed
without max-subtraction (alpha in [-7, 7] for this model family; exp is safe in
fp32). Validated ~4e-3 scale-relative absmax error vs the fp32 reference.
"""

import math
import os
import sys
from dataclasses import dataclass, field

import numpy as np

try:
    import ml_dtypes
except ImportError:  # pragma: no cover
    ml_dtypes = None

for _p in ("/opt/trn_rl_repo", "/root/.axon_site/_ro/trn_rl_repo"):
    if os.path.isdir(_p) and _p not in sys.path:
        sys.path.append(_p)

BF16 = ml_dtypes.bfloat16


@dataclass
class Cfg:
    N: int = 50000          # nodes
    E: int = 800000         # edges
    IN: int = 256           # input feature dim
    C: int = 64             # out channels per head
    ncores: int = 8
    BLK: int = 128          # dst nodes per PSUM block

    @property
    def NPC(self):          # nodes per core
        return self.N // self.ncores

    @property
    def NB(self):           # dst blocks per core
        return (self.NPC + self.BLK - 1) // self.BLK

    @property
    def NPAD(self):         # padded nodes per core
        return self.NB * self.BLK

    @property
    def NALL(self):         # padded nodes total (blocked node space)
        return self.ncores * self.NPAD


# ---------------------------------------------------------------------------
# Host-side prep: edge sorting/sharding + input marshalling (layout only).
# ---------------------------------------------------------------------------

def _np(a):
    return np.asarray(a)


def host_prep(cfg: Cfg, x, edge_index, params):
    N, E, IN, C = cfg.N, cfg.E, cfg.IN, cfg.C
    NPC, BLK, NB, NPAD = cfg.NPC, cfg.BLK, cfg.NB, cfg.NPAD

    x = _np(x).astype(np.float32)
    ei = _np(edge_index)
    src = ei[0].astype(np.int64)
    dst = ei[1].astype(np.int64)

    order = np.argsort(dst, kind="stable")
    ss, ds = src[order], dst[order]
    core_arr = ds // NPC
    loc = ds - core_arr * NPC
    gblk = core_arr * NB + loc // BLK   # non-decreasing
    blk_lo = np.searchsorted(gblk, np.arange(cfg.ncores * NB), side="left")
    blk_hi = np.searchsorted(gblk, np.arange(cfg.ncores * NB), side="right")

    SPLIT = min(32768, cfg.NALL)  # int16 gather-index limit

    def blocked_id(n):
        c = n // NPC
        return c * NPAD + (n - c * NPC)

    # per (core, block): edge lists split into A (blocked src < SPLIT) and B
    eA, eB = {}, {}
    nA = np.zeros((cfg.ncores, NB), np.int64)
    nB = np.zeros((cfg.ncores, NB), np.int64)
    for c in range(cfg.ncores):
        for b in range(NB):
            lo, hi = blk_lo[c * NB + b], blk_hi[c * NB + b]
            sb = blocked_id(ss[lo:hi])
            dd = ds[lo:hi]
            mA = sb < SPLIT
            eA[c, b] = (sb[mA], dd[mA])
            eB[c, b] = (sb[~mA] - SPLIT, dd[~mA])
            nA[c, b] = mA.sum()
            nB[c, b] = (~mA).sum()

    MAs = [int(math.ceil(nA[:, b].max() / BLK)) for b in range(NB)]
    MBs = [int(math.ceil(nB[:, b].max() / BLK)) for b in range(NB)]
    Ms = [max(1, MAs[b] + MBs[b]) for b in range(NB)]
    S = int(sum(Ms))
    moff = np.concatenate([[0], np.cumsum(Ms)[:-1]]).astype(np.int64)
    W16 = int(sum((MAs[b] + MBs[b] + Ms[b]) * 8 for b in range(NB)))

    def wrap16(vals, n_slots):
        """index list -> [128, n_slots/16] int16 (wrapped, group-replicated)"""
        L = np.zeros(n_slots, np.int64)
        L[:len(vals)] = vals
        w = L.reshape(n_slots // 16, 16).T.astype(np.int16)   # [16, n/16]
        return np.tile(w, (8, 1))

    per_core = []
    for c in range(cfg.ncores):
        idx16 = np.zeros((BLK, W16), np.int16)
        rel = np.full((BLK, S), float(BLK), np.float32)
        qloc = np.zeros((BLK, S), np.float64)
        off = 0
        for b in range(NB):
            MA, MB, M = MAs[b], MBs[b], Ms[b]
            sA, dA = eA[c, b]
            sB, dB = eB[c, b]
            if MA:
                idx16[:, off:off + MA * 8] = wrap16(sA, MA * BLK)
                off += MA * 8
            if MB:
                idx16[:, off:off + MB * 8] = wrap16(sB, MB * BLK)
                off += MB * 8
            # q indices + dst_rel follow the same slot order: A then B
            dq = np.concatenate([dA, dB])
            qv = np.zeros(M * BLK, np.int64)
            rv = np.full(M * BLK, float(BLK), np.float32)
            # section A at chunks [0, MA), B at [MA, MA+MB)
            qv[:len(dA)] = dA - c * NPC
            rv[:len(dA)] = dA - (c * NPC + b * BLK)
            if MB:
                qv[MA * BLK:MA * BLK + len(dB)] = dB - c * NPC
                rv[MA * BLK:MA * BLK + len(dB)] = dB - (c * NPC + b * BLK)
            idx16[:, off:off + M * 8] = wrap16(qv, M * BLK)
            off += M * 8
            sl = slice(moff[b], moff[b] + M)
            rel[:, sl] = rv.reshape(M, BLK).T
        assert off == W16
        per_core.append({
            "eidx16": idx16,                  # [128, W16] i16
            "erel": rel.astype(BF16),         # [128, S]
        })

    # x transposed into blocked node space (layout marshalling)
    xT_blk = np.zeros((IN, cfg.NALL), BF16)
    xb = x.astype(BF16)
    for c in range(cfg.ncores):
        xT_blk[:, c * NPAD:c * NPAD + NPC] = xb[c * NPC:(c + 1) * NPC].T
    for c in range(cfg.ncores):
        per_core[c]["xT_own"] = np.ascontiguousarray(
            xT_blk[:, c * NPAD:(c + 1) * NPAD])

    p2, p3, p4 = params["conv2"], params["conv3"], params["conv4"]
    g = params["gn"]
    f32 = np.float32

    def cc(*arrs):
        return np.concatenate([_np(a).astype(f32) for a in arrs], axis=-1)

    shared = {
        "xT_blk": xT_blk,
        "Wx_kv": cc(p2["Wk"], p2["Wv"], p4["Wk"], p4["Wv"]).astype(BF16),
        "bx_kv": cc(p2["bk"], p2["bv"], p4["bk"], p4["bv"])[None].astype(BF16),
        "Wh_kv": cc(p3["Wk"], p3["Wv"]).astype(BF16),
        "bh_kv": cc(p3["bk"], p3["bv"])[None].astype(BF16),
        "Wx_own": cc(p2["Wq"], p2["Ws"], p4["Wq"], p4["Ws"]).astype(BF16),
        "bx_own": cc(p2["bq"], p2["bs"], p4["bq"], p4["bs"])[None].astype(BF16),
        "Wh_own": cc(p3["Wq"], p3["Ws"]).astype(BF16),
        "bh_own": cc(p3["bq"], p3["bs"])[None].astype(BF16),
        "gn_row": cc(g["weight"], g["bias"], g["mean_scale"])[None].astype(f32),
        "iota_row": np.tile(np.arange(BLK, dtype=f32), (BLK, 1)).astype(BF16),
        "vmask_in": (np.arange(BLK) < (NPC - (NB - 1) * BLK)
                     ).astype(f32)[:, None],
    }
    meta = {"Ms": Ms, "MAs": MAs, "MBs": MBs, "S": S, "W16": W16,
            "SPLIT": SPLIT}
    return shared, per_core, meta


# ---------------------------------------------------------------------------
# Kernel builder
# ---------------------------------------------------------------------------

def build(cfg: Cfg, meta, debug=False, skip_edges=False):
    import concourse.bacc as bacc
    import concourse.bass as bass
    import concourse.tile as tile
    import concourse.mybir as mybir
    from concourse.bass import IndirectOffsetOnAxis

    dt = mybir.dt
    Alu = mybir.AluOpType
    Act = mybir.ActivationFunctionType

    N, IN, C = cfg.N, cfg.IN, cfg.C
    BLK, NB, NPAD, NALL = cfg.BLK, cfg.NB, cfg.NPAD, cfg.NALL
    H2 = 2 * C      # 128: conv2 per-node q/k/v width (2 heads)
    KV = 2 * H2     # 256: kv table row width
    NCOL = H2 + 2   # 130: [wv(128) | p(2)] matmul rhs width
    Ms, MAs, MBs = meta["Ms"], meta["MAs"], meta["MBs"]
    S, W16, SPLIT = meta["S"], meta["W16"], meta["SPLIT"]
    moff = np.concatenate([[0], np.cumsum(Ms)[:-1]]).astype(np.int64)
    # per-block offsets into the wrapped int16 index tensor
    i16off = []
    off = 0
    for b in range(NB):
        i16off.append(off)
        off += (MAs[b] + MBs[b] + Ms[b]) * 8
    assert off == W16

    nc = bacc.Bacc("TRN2", target_bir_lowering=False, debug=False,
                   enable_asserts=False, num_devices=cfg.ncores)

    # ---- I/O ----
    f32, bf16, i32 = dt.float32, dt.bfloat16, dt.int32
    xT_blk = nc.dram_tensor("xT_blk", [IN, NALL], bf16, kind="ExternalInput")
    xT_own = nc.dram_tensor("xT_own", [IN, NPAD], bf16, kind="ExternalInput")
    eidx16 = nc.dram_tensor("eidx16", [BLK, W16], dt.int16, kind="ExternalInput")
    erel = nc.dram_tensor("erel", [BLK, S], bf16, kind="ExternalInput")
    Wx_kv = nc.dram_tensor("Wx_kv", [IN, 384], bf16, kind="ExternalInput")
    bx_kv = nc.dram_tensor("bx_kv", [1, 384], bf16, kind="ExternalInput")
    Wh_kv = nc.dram_tensor("Wh_kv", [H2, H2], bf16, kind="ExternalInput")
    bh_kv = nc.dram_tensor("bh_kv", [1, H2], bf16, kind="ExternalInput")
    Wx_own = nc.dram_tensor("Wx_own", [IN, 384], bf16, kind="ExternalInput")
    bx_own = nc.dram_tensor("bx_own", [1, 384], bf16, kind="ExternalInput")
    Wh_own = nc.dram_tensor("Wh_own", [H2, H2], bf16, kind="ExternalInput")
    bh_own = nc.dram_tensor("bh_own", [1, H2], bf16, kind="ExternalInput")
    gn_row = nc.dram_tensor("gn_row", [1, 3 * H2], f32, kind="ExternalInput")
    iota_in = nc.dram_tensor("iota_row", [BLK, BLK], bf16, kind="ExternalInput")
    vmask_in = nc.dram_tensor("vmask_in", [BLK, 1], f32, kind="ExternalInput")
    out_t = nc.dram_tensor("out", [NPAD, C], f32, kind="ExternalOutput")
    if debug:
        dbg_kv2 = nc.dram_tensor("dbg_kv2", [BLK, KV], bf16, kind="ExternalOutput")
        dbg_q2 = nc.dram_tensor("dbg_q2", [BLK, H2], bf16, kind="ExternalOutput")
        dbg_h1 = nc.dram_tensor("dbg_h1", [BLK, NB * H2], f32, kind="ExternalOutput")
        dbg_st = nc.dram_tensor("dbg_st", [1, KV], f32, kind="ExternalOutput")
        dbg_hf = nc.dram_tensor("dbg_hf", [2 * BLK, H2], bf16, kind="ExternalOutput")
        dbg_kv34 = nc.dram_tensor("dbg_kv34", [BLK, KV], bf16, kind="ExternalOutput")
        dbg_q34 = nc.dram_tensor("dbg_q34", [BLK, H2], bf16, kind="ExternalOutput")

    # ---- internal DRAM ----
    kv2_t = nc.dram_tensor("kv2_tbl", [NALL, KV], bf16, kind="Internal")
    kv34_t = nc.dram_tensor("kv34_tbl", [NALL, KV], bf16, kind="Internal")
    q2o_t = nc.dram_tensor("q2_own", [NPAD, H2], bf16, kind="Internal")
    q34o_t = nc.dram_tensor("q34_own", [NPAD, H2], bf16, kind="Internal")
    h_own_t = nc.dram_tensor("h_own", [NPAD, H2], bf16, kind="Internal")
    shared_sp = "Shared" if cfg.ncores > 4 else "Local"
    h_full_t = nc.dram_tensor("h_full", [NALL, H2], bf16, kind="Internal",
                              addr_space=shared_sp)
    st_in_t = nc.dram_tensor("stats_in", [1, KV], f32, kind="Internal")
    st_out_t = nc.dram_tensor("stats_out", [1, KV], f32, kind="Internal",
                              addr_space=shared_sp)

    with tile.TileContext(nc) as tc:
        with (
            tc.tile_pool(name="persist", bufs=1) as pp,
            tc.tile_pool(name="dense", bufs=3) as dp,
            tc.tile_pool(name="edge", bufs=2) as ep,
            tc.tile_pool(name="gath", bufs=3) as gp,
            tc.tile_pool(name="small", bufs=2) as sp,
            tc.tile_pool(name="once", bufs=1) as op_,
            tc.tile_pool(name="dpsum", bufs=3, space="PSUM") as dps,
            tc.tile_pool(name="epsum", bufs=3, space="PSUM") as eps,
            tc.tile_pool(name="spsum", bufs=1, space="PSUM") as sps,
        ):
            # ---- persistent SBUF ----
            s2_own = pp.tile([BLK, NB * H2], f32)      # conv2 skip (own)
            s34_own = pp.tile([BLK, NB * C], f32)      # conv3+conv4 skip (own)
            h1_own = pp.tile([BLK, NB * H2], f32)      # conv2 output (own)
            hmish = pp.tile([BLK, NB * H2], bf16)      # post-norm/mish (own)
            out_sb = pp.tile([BLK, NB * C], f32)       # final output (own)
            eidx_sb = pp.tile([BLK, W16], dt.int16)
            erel_sb = pp.tile([BLK, S], bf16)
            wxkv_sb = pp.tile([BLK, 2 * 384], bf16)    # 2 K-halves side by side
            wxown_sb = pp.tile([BLK, 2 * 384], bf16)
            whkv_sb = pp.tile([BLK, H2], bf16)
            whown_sb = pp.tile([BLK, H2], bf16)
            bias_sb = pp.tile([1, 384 * 2 + H2 * 2], bf16)
            gn_sb = pp.tile([1, 3 * H2], f32)
            iota_sb = pp.tile([BLK, BLK], bf16)
            ones_row = pp.tile([1, BLK], bf16)         # K=1 bias matmul lhsT
            ones_col = pp.tile([BLK, 1], f32)          # stats matmul lhsT
            vmask = pp.tile([BLK, 1], f32)             # last-block valid rows
            affine = pp.tile([1, 2 * H2], f32)         # graphnorm a | b row
            affine_full = pp.tile([BLK, 2 * H2], f32)  # row replicated

            # ---- constants / input staging ----
            nc.sync.dma_start(eidx_sb[:], eidx16[:])
            nc.sync.dma_start(erel_sb[:], erel[:])
            nc.sync.dma_start(wxkv_sb[:, 0:384], Wx_kv[0:BLK, :])
            nc.sync.dma_start(wxkv_sb[:, 384:768], Wx_kv[BLK:IN, :])
            nc.sync.dma_start(wxown_sb[:, 0:384], Wx_own[0:BLK, :])
            nc.sync.dma_start(wxown_sb[:, 384:768], Wx_own[BLK:IN, :])
            nc.sync.dma_start(whkv_sb[:], Wh_kv[:])
            nc.sync.dma_start(whown_sb[:], Wh_own[:])
            nc.sync.dma_start(bias_sb[:, 0:384], bx_kv[:])
            nc.sync.dma_start(bias_sb[:, 384:768], bx_own[:])
            nc.sync.dma_start(bias_sb[:, 768:768 + H2], bh_kv[:])
            nc.sync.dma_start(bias_sb[:, 768 + H2:768 + 2 * H2], bh_own[:])
            nc.sync.dma_start(gn_sb[:], gn_row[:])
            nc.sync.dma_start(iota_sb[:], iota_in[:])
            nc.vector.memset(ones_row[:], 1.0)
            nc.vector.memset(ones_col[:], 1.0)
            nc.sync.dma_start(vmask[:], vmask_in[:])

            b_xkv = bias_sb[:, 0:384]
            b_xown = bias_sb[:, 384:768]
            b_hkv = bias_sb[:, 768:768 + H2]
            b_hown = bias_sb[:, 768 + H2:768 + 2 * H2]

            # ---------------------------------------------------------------
            # Dense pass over x (all nodes): kv2 rows + kv34[k4|v4] half.
            # lhsT = xT tile [128 feat, 128 nodes]; rhs = W [128 feat, 384].
            # ---------------------------------------------------------------
            nblk_all = NALL // BLK
            SUP = 4  # node-tiles per DMA load

            def x_dense(xT_src, n_tiles, W_sb, b_row, sink):
                for st in range(0, n_tiles, SUP):
                    nsub = min(SUP, n_tiles - st)
                    w = nsub * BLK
                    xt = dp.tile([BLK, 2 * SUP * BLK], bf16, tag="xt")
                    nc.sync.dma_start(xt[:, 0:w],
                                      xT_src[0:BLK, st * BLK:st * BLK + w])
                    nc.sync.dma_start(xt[:, SUP * BLK:SUP * BLK + w],
                                      xT_src[BLK:IN, st * BLK:st * BLK + w])
                    for s in range(nsub):
                        t = st + s
                        ps = dps.tile([BLK, 384], f32, space="PSUM", tag="dps")
                        nc.tensor.matmul(ps[:], lhsT=xt[:, s * BLK:(s + 1) * BLK],
                                         rhs=W_sb[:, 0:384],
                                         start=True, stop=False)
                        nc.tensor.matmul(
                            ps[:],
                            lhsT=xt[:, SUP * BLK + s * BLK:SUP * BLK + (s + 1) * BLK],
                            rhs=W_sb[:, 384:768], start=False, stop=False)
                        nc.tensor.matmul(ps[:], lhsT=ones_row[:1, 0:BLK],
                                         rhs=b_row, start=False, stop=True)
                        sink(t, ps)

            def kv_x_sink(t, ps):
                ot = dp.tile([BLK, 384], bf16, tag="dout")
                nc.vector.tensor_copy(ot[:], ps[:])
                nc.sync.dma_start(kv2_t[t * BLK:(t + 1) * BLK, :], ot[:, 0:KV])
                nc.sync.dma_start(kv34_t[t * BLK:(t + 1) * BLK, H2:KV],
                                  ot[:, KV:384])

            x_dense(xT_blk, nblk_all, wxkv_sb, b_xkv[:1, :], kv_x_sink)

            # ---- own dense pass over x: q2 | s2 | q4 | s4 ----
            def own_x_sink(b, ps):
                qb = dp.tile([BLK, H2 + C], bf16, tag="qbf")
                nc.vector.tensor_copy(qb[:, 0:H2], ps[:, 0:H2])
                nc.vector.tensor_copy(qb[:, H2:H2 + C], ps[:, KV:KV + C])
                nc.sync.dma_start(q2o_t[b * BLK:(b + 1) * BLK, :], qb[:, 0:H2])
                nc.sync.dma_start(q34o_t[b * BLK:(b + 1) * BLK, C:H2],
                                  qb[:, H2:H2 + C])
                nc.vector.tensor_copy(s2_own[:, b * H2:(b + 1) * H2],
                                      ps[:, H2:KV])
                nc.vector.tensor_copy(s34_own[:, b * C:(b + 1) * C],
                                      ps[:, KV + C:384])

            x_dense(xT_own, NB, wxown_sb, b_xown[:1, :], own_x_sink)

            # ---------------------------------------------------------------
            # Edge pass (shared for conv2 and conv34)
            # ---------------------------------------------------------------
            def edge_pass(kv_tbl, q_tbl, conv2_layout, epilogue):
                for b in range(NB):
                    M, MA, MB = int(Ms[b]), int(MAs[b]), int(MBs[b])
                    mo = int(moff[b])
                    io = i16off[b]
                    kv_t_sb = gp.tile([BLK, M * KV], bf16, tag="kvg")
                    qg = ep.tile([BLK, M * H2], bf16, tag="qg")
                    if MA:
                        nc.gpsimd.dma_gather(
                            kv_t_sb[:, 0:MA * KV].rearrange(
                                "p (m w) -> p m w", w=KV),
                            kv_tbl[0:SPLIT, :],
                            eidx_sb[:, io:io + MA * 8],
                            MA * BLK, MA * BLK, KV, single_packet=False)
                        io += MA * 8
                    if MB:
                        nc.gpsimd.dma_gather(
                            kv_t_sb[:, MA * KV:M * KV].rearrange(
                                "p (m w) -> p m w", w=KV),
                            kv_tbl[SPLIT:NALL, :],
                            eidx_sb[:, io:io + MB * 8],
                            MB * BLK, MB * BLK, KV, single_packet=False)
                        io += MB * 8
                    nc.gpsimd.dma_gather(
                        qg[:].rearrange("p (m w) -> p m w", w=H2),
                        q_tbl[:],
                        eidx_sb[:, io:io + M * 8],
                        M * BLK, M * BLK, H2, single_packet=False)
                    kv4 = kv_t_sb[:].rearrange("p (m w) -> p m w", w=KV)
                    if conv2_layout:
                        # kv row = [k(2x64) | v(2x64)] (head stride C)
                        k_ap = kv4[:, :, 0:H2].rearrange(
                            "p m (h c) -> p m h c", c=C)
                        v_ap = kv4[:, :, H2:KV].rearrange(
                            "p m (h c) -> p m h c", c=C)
                    else:
                        # kv row = [k3|v3|k4|v4] (head stride H2)
                        kvh = kv4.rearrange("p m (h g) -> p m h g", h=2)
                        k_ap = kvh[:, :, :, 0:C]
                        v_ap = kvh[:, :, :, C:H2]
                    # prod = q * k  -> [128, M, 2, C]
                    prod = ep.tile([BLK, M * H2], bf16, tag="prod")
                    nc.vector.tensor_mul(
                        prod[:].rearrange("p (m h c) -> p m h c", h=2, c=C),
                        qg[:].rearrange("p (m h c) -> p m h c", h=2, c=C),
                        k_ap)
                    # alpha = rowsum over C -> [128, 2M] f32
                    alpha = ep.tile([BLK, M * 2], f32, tag="alpha")
                    nc.vector.reduce_sum(
                        alpha[:],
                        prod[:].rearrange("p (mh c) -> p mh c", c=C),
                        axis=mybir.AxisListType.X)
                    # p = exp(alpha/8) into wvp[:, :, 128:130]
                    wvp = ep.tile([BLK, M * NCOL], bf16, tag="wvp")
                    wvp3 = wvp[:].rearrange("p (m w) -> p m w", w=NCOL)
                    nc.scalar.activation(wvp3[:, :, H2:NCOL], alpha[:].rearrange(
                        "p (m h) -> p m h", h=2), Act.Exp,
                        scale=float(1.0 / math.sqrt(C)))
                    # wv = v * p
                    p_ap = wvp3[:, :, H2:NCOL].unsqueeze(3).broadcast_to(
                        [BLK, M, 2, C])
                    nc.vector.tensor_mul(
                        wvp3[:, :, 0:H2].rearrange("p m (h c) -> p m h c", c=C),
                        v_ap, p_ap)
                    # onehot[e, n] = (dst_rel[e] == n)
                    oh = ep.tile([BLK, M * BLK], bf16, tag="oh")
                    rel_ap = erel_sb[:, mo:mo + M].unsqueeze(2).broadcast_to(
                        [BLK, M, BLK])
                    io_ap = iota_sb[:].unsqueeze(1).broadcast_to(
                        [BLK, M, BLK])
                    nc.vector.tensor_tensor(
                        oh[:].rearrange("p (m n) -> p m n", n=BLK),
                        rel_ap, io_ap, op=Alu.is_equal)
                    # segment accumulate: psum[n, :] += onehot^T @ [wv | p]
                    ps = eps.tile([BLK, NCOL], f32, space="PSUM", tag="eps")
                    for j in range(M):
                        nc.tensor.matmul(
                            ps[:], lhsT=oh[:, j * BLK:(j + 1) * BLK],
                            rhs=wvp[:, j * NCOL:(j + 1) * NCOL],
                            start=(j == 0), stop=(j == M - 1))
                    epilogue(b, ps)

            # ---- conv2 edge pass ----
            st_ps = sps.tile([1, H2], f32, space="PSUM", tag="sth")
            st_ps2 = sps.tile([1, H2], f32, space="PSUM", tag="stsq")

            def epi_conv2(b, ps):
                den = sp.tile([BLK, 2], f32, tag="den")
                nc.vector.tensor_scalar_add(den[:], ps[:, H2:NCOL], 1e-16)
                rden = sp.tile([BLK, 2], f32, tag="rden")
                nc.vector.reciprocal(rden[:], den[:])
                h1b = h1_own[:, b * H2:(b + 1) * H2]
                rd_ap = rden[:].unsqueeze(2).broadcast_to([BLK, 2, C])
                nc.vector.tensor_mul(
                    h1b.rearrange("p (h c) -> p h c", c=C),
                    ps[:, 0:H2].rearrange("p (h c) -> p h c", c=C), rd_ap)
                nc.vector.tensor_add(h1b, h1b, s2_own[:, b * H2:(b + 1) * H2])
                if b == NB - 1:
                    nc.vector.tensor_scalar_mul(h1b, h1b, vmask[:, 0:1])
                sq = sp.tile([BLK, H2], f32, tag="sq")
                nc.scalar.square(sq[:], h1b)
                nc.tensor.matmul(st_ps[0:1, :], lhsT=ones_col[:, 0:1],
                                 rhs=h1b, start=(b == 0), stop=(b == NB - 1))
                nc.tensor.matmul(st_ps2[0:1, :], lhsT=ones_col[:, 0:1],
                                 rhs=sq[:], start=(b == 0), stop=(b == NB - 1))

            if not skip_edges:
                edge_pass(kv2_t, q2o_t, conv2_layout=True,
                          epilogue=epi_conv2)
            else:
                for b in range(NB):
                    epi_dummy_ps = eps.tile([BLK, NCOL], f32, space="PSUM",
                                            tag="eps")
                    nc.tensor.matmul(epi_dummy_ps[:], lhsT=iota_sb[:],
                                     rhs=wxkv_sb[:, 0:NCOL], start=True,
                                     stop=True)
                    epi_conv2(b, epi_dummy_ps)

            # ---- GraphNorm stats allreduce + affine ----
            st_sb = op_.tile([1, KV], f32, tag="st")
            nc.vector.tensor_copy(st_sb[:1, 0:H2], st_ps[:])
            nc.vector.tensor_copy(st_sb[:1, H2:KV], st_ps2[:])
            nc.sync.dma_start(st_in_t[:], st_sb[:])
            nc.gpsimd.collective_compute(
                "AllReduce", Alu.add,
                replica_groups=[list(range(cfg.ncores))],
                ins=[st_in_t[:].opt()], outs=[st_out_t[:].opt()])
            stg = op_.tile([1, KV], f32, tag="stg")
            nc.sync.dma_start(stg[:], st_out_t[:])
            mean = op_.tile([1, H2], f32, tag="mean")
            nc.vector.tensor_scalar_mul(mean[:], stg[:1, 0:H2], 1.0 / N)
            e2 = op_.tile([1, H2], f32, tag="e2")
            nc.vector.tensor_scalar_mul(e2[:], stg[:1, H2:KV], 1.0 / N)
            m2 = op_.tile([1, H2], f32, tag="m2")
            nc.vector.tensor_mul(m2[:], mean[:], gn_sb[:1, 2 * H2:3 * H2])
            # var = e2 - 2*mean*m2 + m2^2
            var = op_.tile([1, H2], f32, tag="var")
            t1 = op_.tile([1, H2], f32, tag="t1")
            nc.vector.tensor_mul(t1[:], mean[:], m2[:])
            nc.vector.scalar_tensor_tensor(var[:], t1[:], -2.0, e2[:],
                                           op0=Alu.mult, op1=Alu.add)
            nc.vector.tensor_mul(t1[:], m2[:], m2[:])
            nc.vector.tensor_add(var[:], var[:], t1[:])
            nc.vector.tensor_scalar_add(var[:], var[:], 1e-5)
            sd = op_.tile([1, H2], f32, tag="sd")
            nc.scalar.activation(sd[:], var[:], Act.Sqrt)
            rsd = op_.tile([1, H2], f32, tag="rsd")
            nc.vector.reciprocal(rsd[:], sd[:])
            nc.vector.tensor_mul(affine[:1, 0:H2], rsd[:],
                                 gn_sb[:1, 0:H2])             # a
            nc.vector.tensor_mul(t1[:], affine[:1, 0:H2], m2[:])
            nc.vector.tensor_sub(affine[:1, H2:KV],
                                 gn_sb[:1, H2:KV], t1[:])     # b

            # ---- normalize + mish over own nodes ----
            ones_f32 = op_.tile([1, BLK], f32, tag="of")
            nc.vector.memset(ones_f32[:], 1.0)
            ps_aff = dps.tile([BLK, 384], f32, space="PSUM", tag="dps")
            nc.tensor.matmul(ps_aff[:, 0:KV], lhsT=ones_f32[:1, 0:BLK],
                             rhs=affine[:1, :], start=True, stop=True)
            nc.vector.tensor_copy(affine_full[:], ps_aff[:, 0:KV])
            a_ap = affine_full[:, 0:H2].unsqueeze(1).broadcast_to(
                [BLK, NB, H2])
            b_ap = affine_full[:, H2:KV].unsqueeze(1).broadcast_to(
                [BLK, NB, H2])
            h3 = h1_own[:].rearrange("p (b w) -> p b w", w=H2)
            nc.vector.tensor_mul(h3, h3, a_ap)
            nc.vector.tensor_add(h3, h3, b_ap)
            # mish(x) = x * tanh(ln(1 + exp(x)))  (no Mish/Softplus LUT on HW)
            nc.scalar.activation(hmish[:], h1_own[:], Act.Exp)
            nc.scalar.activation(hmish[:], hmish[:], Act.Ln, bias=1.0)
            nc.scalar.activation(hmish[:], hmish[:], Act.Tanh)
            nc.vector.tensor_tensor(hmish[:], h1_own[:], hmish[:], op=Alu.mult)
            nc.vector.tensor_scalar_mul(
                hmish[:, (NB - 1) * H2:NB * H2],
                hmish[:, (NB - 1) * H2:NB * H2], vmask[:, 0:1])
            # write h_own (blocked node-major) + allgather
            nc.sync.dma_start(
                h_own_t[:].rearrange("(b p) w -> p b w", p=BLK),
                hmish[:].rearrange("p (b w) -> p b w", w=H2))
            nc.gpsimd.collective_compute(
                "AllGather", Alu.bypass,
                replica_groups=[list(range(cfg.ncores))],
                ins=[h_own_t[:].opt()], outs=[h_full_t[:].opt()])

            # ---- own dense pass over h: q3 | s3 ----
            for b in range(NB):
                ht = dp.tile([BLK, BLK], bf16, tag="ht")
                nc.sync.dma_start_transpose(
                    ht[:], h_own_t[b * BLK:(b + 1) * BLK, :])
                ps = dps.tile([BLK, 384], f32, space="PSUM", tag="dps")
                nc.tensor.matmul(ps[:, 0:H2], lhsT=ht[:], rhs=whown_sb[:],
                                 start=True, stop=False)
                nc.tensor.matmul(ps[:, 0:H2], lhsT=ones_row[:1, 0:BLK],
                                 rhs=b_hown[:1, :], start=False, stop=True)
                qb = dp.tile([BLK, H2 + C], bf16, tag="qbf")
                nc.vector.tensor_copy(qb[:, 0:C], ps[:, 0:C])
                nc.sync.dma_start(q34o_t[b * BLK:(b + 1) * BLK, 0:C],
                                  qb[:, 0:C])
                nc.vector.tensor_add(s34_own[:, b * C:(b + 1) * C],
                                     s34_own[:, b * C:(b + 1) * C],
                                     ps[:, C:H2])

            # ---- dense pass over h (all nodes): kv34 [k3|v3] half ----
            for t in range(nblk_all):
                ht = dp.tile([BLK, BLK], bf16, tag="ht")
                nc.sync.dma_start_transpose(
                    ht[:], h_full_t[t * BLK:(t + 1) * BLK, :])
                ps = dps.tile([BLK, 384], f32, space="PSUM", tag="dps")
                nc.tensor.matmul(ps[:, 0:H2], lhsT=ht[:], rhs=whkv_sb[:],
                                 start=True, stop=False)
                nc.tensor.matmul(ps[:, 0:H2], lhsT=ones_row[:1, 0:BLK],
                                 rhs=b_hkv[:1, :], start=False, stop=True)
                ot = dp.tile([BLK, 384], bf16, tag="dout")
                nc.vector.tensor_copy(ot[:, 0:H2], ps[:, 0:H2])
                nc.sync.dma_start(kv34_t[t * BLK:(t + 1) * BLK, 0:H2],
                                  ot[:, 0:H2])

            # ---- conv3+conv4 fused edge pass ----
            def epi_conv34(b, ps):
                den = sp.tile([BLK, 2], f32, tag="den")
                nc.vector.tensor_scalar_add(den[:], ps[:, H2:NCOL], 1e-16)
                rden = sp.tile([BLK, 2], f32, tag="rden")
                nc.vector.reciprocal(rden[:], den[:])
                tt = sp.tile([BLK, H2], f32, tag="tt")
                rd_ap = rden[:].unsqueeze(2).broadcast_to([BLK, 2, C])
                nc.vector.tensor_mul(
                    tt[:].rearrange("p (h c) -> p h c", c=C),
                    ps[:, 0:H2].rearrange("p (h c) -> p h c", c=C), rd_ap)
                ob = out_sb[:, b * C:(b + 1) * C]
                nc.vector.tensor_add(ob, tt[:, 0:C], tt[:, C:H2])
                nc.vector.tensor_add(ob, ob, s34_own[:, b * C:(b + 1) * C])

            if not skip_edges:
                edge_pass(kv34_t, q34o_t, conv2_layout=False,
                          epilogue=epi_conv34)
            else:
                for b in range(NB):
                    epi_dummy_ps = eps.tile([BLK, NCOL], f32, space="PSUM",
                                            tag="eps")
                    nc.tensor.matmul(epi_dummy_ps[:], lhsT=iota_sb[:],
                                     rhs=wxkv_sb[:, 0:NCOL], start=True,
                                     stop=True)
                    epi_conv34(b, epi_dummy_ps)

            if debug:
                nc.sync.dma_start(dbg_kv2[:], kv2_t[0:BLK, :])
                nc.sync.dma_start(dbg_q2[:], q2o_t[0:BLK, :])
                nc.sync.dma_start(dbg_h1[:], h1_own[:])
                nc.sync.dma_start(dbg_st[:], stg[:])
                nc.sync.dma_start(dbg_hf[:], h_full_t[0:2 * BLK, :])
                nc.sync.dma_start(dbg_kv34[:], kv34_t[0:BLK, :])
                nc.sync.dma_start(dbg_q34[:], q34o_t[0:BLK, :])
            # ---- final output ----
            nc.sync.dma_start(
                out_t[:].rearrange("(b p) w -> p b w", p=BLK),
                out_sb[:].rearrange("p (b w) -> p b w", w=C))

    nc.compile()
    return nc


# ---------------------------------------------------------------------------
# Entry point
# ---------------------------------------------------------------------------

def kernel(x=None, edge_index=None, params=None, _bench=False, **kw):
    if x is None:
        x = kw["x"]
    if edge_index is None:
        edge_index = kw["edge_index"]
    if params is None:
        params = kw["params"]
    cfg = Cfg()
    params = {k: {kk: _np(vv) for kk, vv in v.items()}
              for k, v in params.items()}
    shared, per_core, meta = host_prep(cfg, x, edge_index, params)
    nc = build(cfg, meta)

    from concourse import bass_utils
    in_maps = []
    for c in range(cfg.ncores):
        m = dict(shared)
        m.update(per_core[c])
        in_maps.append(m)
    res = bass_utils.run_bass_kernel_spmd(
        nc, in_maps, core_ids=list(range(cfg.ncores)))
    outs = [res.results[c]["out"][:cfg.NPC] for c in range(cfg.ncores)]
    full = np.concatenate(outs, axis=0).astype(np.float32)
    if _bench:
        return full, res
    return full


if __name__ == "__main__":
    import reference
    inputs = reference.setup_inputs()
    out = kernel(**{k: v for k, v in inputs.items()})
    print("out", out.shape, out.dtype)
